# revision 5
# baseline (speedup 1.0000x reference)
"""Trainium2 Bass kernel for nn_AttractorLayerUnnormed.

Computes, for full inputs
    x [4,256,96,128], b_prev [4,64,48,64], w1 [128,256], b1 [128],
    w2 [16,128], b2 [16]:
  hid = relu(w1 @ x + b1)                    (1x1 conv)
  A   = softplus(w2 @ hid + b2)              [n, 16, 96, 128]
  b_c = bilinear_resize(b_prev, 96, 128)     (align_corners) [n, 64, 96, 128]
  out = b_c + sum_a (A_a - b_c) * exp(-300 (A_a - b_c)^2)

Sharding: 8 cores = (sample n in 0..3) x (h-half in 0..1); each core owns
48 rows x 128 cols = 6144 positions.

Per-core device program (12 chunks of F=512 positions):
  - mm1 (K=256 via 2 matmuls) + ReLU          -> hid [128, F]
  - mm2 (K=128, M=16) + softplus (Exp+Ln)     -> A_t [16, F]
  - bilinear resize as one matmul per output row:
      lhsT = Bsel[:, y, :] [128=(j,l), 64bin], rhs = Wy[y] [128, 128x]
    (host pre-gathers the two source rows per output row and pre-scales
     the column-interp matrix by the row weights)
  - attractor loop over j in 0..7 (8 bins at a time, partitions=(g, a)):
      dx   = Asel.T @ A_t - BselJ[j].T @ b_tile   (PE, PSUM accumulate)
      sq   = 300*dx^2            (DVE or ACT, alternating for balance)
      e    = exp(-sq)            (ACT)
      term = dx * e              (DVE)
      delta += SselJ[j].T @ term (PE, PSUM accumulate over j)
  - out_tile = delta + b_tile -> DMA to DRAM
"""

import numpy as np

import concourse.bacc as bacc
import concourse.tile as tile
from concourse import mybir
from concourse.bass_utils import run_bass_kernel_spmd

ALPHA = 300.0
N_CORES = 8
S = 48 * 128  # positions per core
NCHUNK = 12
F = 512  # positions per chunk
SQRT_A = float(np.sqrt(ALPHA))

# which j-iterations compute sq on DVE (rest on ACT) - load balance knob
DVE_SQ_JS = (0, 2, 5)

_CACHE = {}


def _f32(x):
    return np.ascontiguousarray(x, dtype=np.float32)


def _host_prep(inputs):
    x = np.asarray(inputs["x"], dtype=np.float32)
    b_prev = np.asarray(inputs["b_prev"], dtype=np.float32)
    w1 = np.asarray(inputs["w1"], dtype=np.float32)
    b1 = np.asarray(inputs["b1"], dtype=np.float32)
    w2 = np.asarray(inputs["w2"], dtype=np.float32)
    b2 = np.asarray(inputs["b2"], dtype=np.float32)

    H, W, h_in, w_in = 96, 128, 48, 64

    ys = np.linspace(0.0, h_in - 1.0, H)
    y0 = np.floor(ys).astype(np.int64)
    wy = (ys - y0).astype(np.float32)
    xs_ = np.linspace(0.0, w_in - 1.0, W)
    x0 = np.floor(xs_).astype(np.int64)
    x1 = np.minimum(x0 + 1, w_in - 1)
    wx = (xs_ - x0).astype(np.float32)

    CxT = np.zeros((w_in, W), dtype=np.float32)
    CxT[x0, np.arange(W)] += 1.0 - wx
    CxT[x1, np.arange(W)] += wx

    per_core = []
    for core in range(N_CORES):
        n, half = core // 2, core % 2
        h0 = half * 48
        y0l = y0[h0 : h0 + 48]
        wyl = wy[h0 : h0 + 48]

        xs_c = _f32(x[n, :, h0 : h0 + 48, :].reshape(2, 128, S))

        bp_t = b_prev[n].transpose(2, 1, 0)  # [l, k, bin]
        Bsel = np.empty((2, 64, 48, 64), dtype=np.float32)
        for j in range(2):
            Bsel[j] = bp_t[:, np.clip(y0l + j, 0, 47), :]
        Bsel = _f32(Bsel.reshape(128, 48, 64))

        Wy = np.empty((48, 128, W), dtype=np.float32)
        Wy[:, :64, :] = (1.0 - wyl)[:, None, None] * CxT[None]
        Wy[:, 64:, :] = wyl[:, None, None] * CxT[None]
        Wy = Wy.transpose(1, 0, 2)  # [128, 48, 128], partition-major

        per_core.append({"xs": xs_c, "bsel": Bsel, "wy": _f32(Wy)})

    m = np.arange(128)
    consts = {
        "w1t": _f32(w1.T.reshape(2, 128, 128)),
        "w2t": _f32(w2.T),  # [128, 16]
        "b1": _f32(b1.reshape(128, 1)),
        "b2": _f32(np.concatenate([b2, np.zeros(112, np.float32)]).reshape(128, 1)),
        "asel": _f32(np.arange(16)[:, None] == (m[None, :] % 16)),  # [16, 128]
        "nbselj": _f32(
            -np.stack(
                [
                    (np.arange(64)[:, None] == (8 * j + m[None, :] // 16)).astype(
                        np.float32
                    )
                    for j in range(8)
                ],
                axis=1,
            )
        ),  # [64, 8, 128]
        "sselj": _f32(
            np.stack(
                [
                    ((8 * j + m[:, None] // 16) == np.arange(64)[None, :])
                    for j in range(8)
                ],
                axis=1,
            )
        ),  # [128, 8, 64]
        "ones": np.ones((128, 1), dtype=np.float32),
    }
    return per_core, consts


def _build_bass():
    nc = bacc.Bacc(None, target_bir_lowering=False)
    dt = mybir.dt.float32
    AF = mybir.ActivationFunctionType
    OP = mybir.AluOpType

    xs = nc.dram_tensor("xs", [2, 128, S], dt, kind="ExternalInput")
    bsel = nc.dram_tensor("bsel", [128, 48, 64], dt, kind="ExternalInput")
    wy = nc.dram_tensor("wy", [128, 48, 128], dt, kind="ExternalInput")
    w1t = nc.dram_tensor("w1t", [2, 128, 128], dt, kind="ExternalInput")
    w2t = nc.dram_tensor("w2t", [128, 16], dt, kind="ExternalInput")
    b1 = nc.dram_tensor("b1", [128, 1], dt, kind="ExternalInput")
    b2 = nc.dram_tensor("b2", [128, 1], dt, kind="ExternalInput")
    asel = nc.dram_tensor("asel", [16, 128], dt, kind="ExternalInput")
    nbselj = nc.dram_tensor("nbselj", [64, 8, 128], dt, kind="ExternalInput")
    sselj = nc.dram_tensor("sselj", [128, 8, 64], dt, kind="ExternalInput")
    ones = nc.dram_tensor("ones", [128, 1], dt, kind="ExternalInput")
    out = nc.dram_tensor("out", [64, 48, 128], dt, kind="ExternalOutput")

    with tile.TileContext(nc) as tc:
        with (
            tc.tile_pool(name="singles", bufs=1) as singles,
            tc.tile_pool(name="xin", bufs=3) as xin,
            tc.tile_pool(name="work", bufs=2) as work,
            tc.tile_pool(name="small", bufs=2) as small,
            tc.tile_pool(name="jwork", bufs=3) as jwork,
            tc.tile_pool(name="ph", bufs=1, space="PSUM") as ph,
            tc.tile_pool(name="pz", bufs=1, space="PSUM") as pz,
            tc.tile_pool(name="pb", bufs=1, space="PSUM") as pb,
            tc.tile_pool(name="pdx", bufs=2, space="PSUM") as pdx,
            tc.tile_pool(name="pd", bufs=2, space="PSUM") as pd,
        ):
            # resident weights / constants
            w1t_sb = singles.tile([128, 2, 128], dt)
            nc.sync.dma_start(out=w1t_sb[:, 0, :], in_=w1t[0])
            nc.sync.dma_start(out=w1t_sb[:, 1, :], in_=w1t[1])
            w2t_sb = singles.tile([128, 16], dt)
            nc.sync.dma_start(out=w2t_sb, in_=w2t[:, :])
            b1_sb = singles.tile([128, 1], dt)
            nc.sync.dma_start(out=b1_sb, in_=b1[:, :])
            b2_sb = singles.tile([128, 1], dt)
            nc.sync.dma_start(out=b2_sb, in_=b2[:, :])
            ones_sb = singles.tile([128, 1], dt)
            nc.sync.dma_start(out=ones_sb, in_=ones[:, :])
            asel_sb = singles.tile([16, 128], dt)
            nc.sync.dma_start(out=asel_sb, in_=asel[:, :])
            nbsel_sb = singles.tile([64, 8, 128], dt)
            nc.sync.dma_start(out=nbsel_sb, in_=nbselj[:, :, :])
            ssel_sb = singles.tile([128, 8, 64], dt)
            nc.sync.dma_start(out=ssel_sb, in_=sselj[:, :, :])
            bsel_sb = singles.tile([128, 48, 64], dt)
            nc.sync.dma_start(out=bsel_sb, in_=bsel[:, :, :])
            wy_sb = singles.tile([128, 48, 128], dt)
            nc.sync.dma_start(out=wy_sb, in_=wy[:, :, :])

            for c in range(NCHUNK):
                sl = slice(c * F, (c + 1) * F)
                # ---- mm1 + relu ----
                x0t = xin.tile([128, F], dt, tag="xt")
                x1t = xin.tile([128, F], dt, tag="xt")
                nc.sync.dma_start(out=x0t, in_=xs[0, :, sl])
                nc.sync.dma_start(out=x1t, in_=xs[1, :, sl])
                psum_h = ph.tile([128, F], dt)
                nc.tensor.matmul(psum_h, w1t_sb[:, 0, :], x0t, start=True, stop=False)
                nc.tensor.matmul(psum_h, w1t_sb[:, 1, :], x1t, start=False, stop=True)
                hid = work.tile([128, F], dt, tag="hid")
                nc.scalar.activation(hid, psum_h, AF.Relu, bias=b1_sb[:, 0:1])

                # ---- mm2 + softplus (Exp then Ln(1+x)) ----
                psum_z = pz.tile([16, F], dt)
                nc.tensor.matmul(psum_z, w2t_sb, hid, start=True, stop=True)
                ez = small.tile([16, F], dt, tag="ez")
                nc.scalar.activation(ez, psum_z, AF.Exp, bias=b2_sb[:16, 0:1])
                a_t = small.tile([16, F], dt, tag="at")
                nc.scalar.activation(a_t, ez, AF.Ln, bias=ones_sb[:16, 0:1])

                # ---- bilinear resize: 4 output rows per chunk ----
                psum_b = pb.tile([64, 4, 128], dt)
                for yl in range(4):
                    y = 4 * c + yl
                    nc.tensor.matmul(
                        psum_b[:, yl, :],
                        bsel_sb[:, y, :],
                        wy_sb[:, y, :],
                        start=True,
                        stop=True,
                    )
                b_tile = work.tile([64, F], dt, tag="bt")
                nc.scalar.activation(
                    b_tile, psum_b[:, :, :].rearrange("p a b -> p (a b)"), AF.Copy
                )

                # ---- attractor loop ----
                psum_d = pd.tile([64, F], dt)
                for j in range(8):
                    psum_dx = pdx.tile([128, F], dt)
                    nc.tensor.matmul(psum_dx, asel_sb, a_t, start=True, stop=False)
                    nc.tensor.matmul(
                        psum_dx, nbsel_sb[:, j, :], b_tile, start=False, stop=True
                    )
                    sq = jwork.tile([128, F], dt, tag="sq")
                    term = jwork.tile([128, F], dt, tag="tm")
                    e_t = jwork.tile([128, F], dt, tag="et")
                    if j in DVE_SQ_JS:
                        # DVE-heavy variant: copy dx to SBUF (2x mode), square
                        # and multiply on DVE; ACT only does the exp.
                        dxs = jwork.tile([128, F], dt, tag="dxs")
                        nc.vector.tensor_copy(dxs, psum_dx)
                        nc.vector.scalar_tensor_tensor(
                            sq, dxs, ALPHA, dxs, op0=OP.mult, op1=OP.mult
                        )
                        nc.scalar.activation(e_t, sq, AF.Exp, scale=-1.0)
                        nc.vector.tensor_tensor(term, dxs, e_t, op=OP.mult)
                    else:
                        nc.scalar.activation(sq, psum_dx, AF.Square, scale=SQRT_A)
                        nc.scalar.activation(e_t, sq, AF.Exp, scale=-1.0)
                        nc.vector.tensor_tensor(term, psum_dx, e_t, op=OP.mult)
                    nc.tensor.matmul(
                        psum_d,
                        ssel_sb[:, j, :],
                        term,
                        start=(j == 0),
                        stop=(j == 7),
                    )

                # ---- final add + store ----
                out_t = work.tile([64, F], dt, tag="ot")
                nc.vector.tensor_add(out_t, psum_d, b_tile)
                nc.sync.dma_start(
                    out=out[:, 4 * c : 4 * c + 4, :],
                    in_=out_t[:, :].rearrange("p (a b) -> p a b", a=4),
                )

    nc.compile()
    return nc


def _get_nc():
    if "nc" not in _CACHE:
        _CACHE["nc"] = _build_bass()
    return _CACHE["nc"]


def kernel(**inputs):
    nc = _get_nc()
    per_core, consts = _host_prep(inputs)
    in_maps = [dict(consts, **pc) for pc in per_core]
    res = run_bass_kernel_spmd(nc, in_maps, core_ids=list(range(N_CORES)))
    out = np.zeros((4, 64, 96, 128), dtype=np.float32)
    for core in range(N_CORES):
        n, half = core // 2, core % 2
        out[n, :, half * 48 : half * 48 + 48, :] = res.results[core]["out"]
    return out
